# revision 41
# baseline (speedup 1.0000x reference)
import sys

sys.path.insert(0, "/opt/trn_rl_repo")

import ml_dtypes
import numpy as np

import concourse.bass as bass
import concourse.mybir as mybir
from concourse.bass_utils import run_bass_kernel_spmd
from concourse.tile import TileContext


# Problem constants (nn_CRFLoss: B,V,S,K = 16,8,256,66 on 8 cores)
B, V, S, K = 16, 8, 256, 66
NCORES = 8
BPC = B // NCORES          # batches per core (2)
NPC = BPC * V              # CRF sequences per core (16)
ROWS = BPC * S * S         # argmax rows per core (131072)
WTILE = 64                 # segments (rows) per partition per argmax tile
NTILES = ROWS // (128 * WTILE)  # 16
GSHIFT = 5.0               # per-step log-scale taken out of the emissions
RENORM = 16                # renormalize the linear recurrence every RENORM steps

_CACHED = None


def _split_multi_waits(nc):
    """This walrus build accepts at most one semaphore wait per compute
    instruction. Move extra waits onto standalone EventSemaphore
    instructions inserted just before, on the same engine stream."""
    n = 0
    for f in nc.m.functions:
        for blk in f.blocks:
            out = []
            for inst in blk.instructions:
                si = getattr(inst, "sync_info", None)
                waits = list(si.on_wait) if si is not None and si.on_wait else []
                if len(waits) > 1:
                    for w in waits[:-1]:
                        ev = mybir.InstEventSemaphore(
                            name=f"I-evsplit-{n}", ins=[], outs=[]
                        )
                        n += 1
                        ev.engine = inst.engine
                        ev.sync_info = mybir.SyncInfo(on_wait=[w], on_update=[])
                        out.append(ev)
                    si.on_wait = waits[-1:]
                out.append(inst)
            blk.instructions = out


def _build_bass():
    f32 = mybir.dt.float32
    u32 = mybir.dt.uint32
    i32 = mybir.dt.int32
    Alu = mybir.AluOpType
    Act = mybir.ActivationFunctionType

    nc = bass.Bass()
    EW = 97  # lhsT width: 66 transition cols, zero pad, exp(end) at col 96
    bf16 = mybir.dt.bfloat16
    lp = nc.declare_dram_parameter("lp", [NTILES, 128, WTILE, K], f32, isOutput=False)
    elt = nc.declare_dram_parameter("elt", [K, S - 1, NPC], bf16, isOutput=False)
    a0 = nc.declare_dram_parameter("a0", [K, NPC], bf16, isOutput=False)
    eaug = nc.declare_dram_parameter("eaug", [K, EW], bf16, isOutput=False)
    ones1 = nc.declare_dram_parameter("ones1", [1, K], f32, isOutput=False)
    basec = nc.declare_dram_parameter("basec", [128, WTILE], u32, isOutput=False)
    pred_o = nc.declare_dram_parameter("pred", [NTILES, 128, WTILE], i32, isOutput=True)
    z_o = nc.declare_dram_parameter("z", [1, S * NPC], f32, isOutput=True)

    with TileContext(nc) as tc:
        with (
            tc.tile_pool(name="consts", bufs=1) as consts,
            tc.tile_pool(name="xin", bufs=4) as xin,
            tc.tile_pool(name="mstats", bufs=3) as mstats,
            tc.tile_pool(name="preds", bufs=3) as preds,
            tc.tile_pool(name="astate", bufs=3) as astate,
            tc.tile_pool(name="small", bufs=4) as small,
            tc.tile_pool(name="zrows", bufs=1) as zrows,
            tc.tile_pool(name="ps", bufs=2, space="PSUM") as ps,
            tc.tile_pool(name="psb", bufs=2, space="PSUM") as psb,
        ):
            # ---- constants ----
            # PE-visible tiles are produced by DVE copies so that every
            # Matmult collapses to a single (DVE) semaphore wait — this
            # walrus build rejects Matmults with more than one wait.
            sb_elt = consts.tile([K, S - 1, NPC], bf16)
            nc.sync.dma_start(out=sb_elt, in_=elt[:])
            sb_base = consts.tile([128, WTILE], u32)
            nc.sync.dma_start(out=sb_base, in_=basec[:])

            st_eaug = consts.tile([K, EW], bf16, tag="st_eaug")
            nc.sync.dma_start(out=st_eaug, in_=eaug[:])
            st_ones = consts.tile([1, K], f32, tag="st_ones")
            nc.sync.dma_start(out=st_ones, in_=ones1[:])
            st_a0 = consts.tile([K, NPC], bf16, tag="st_a0")
            nc.sync.dma_start(out=st_a0, in_=a0[:])

            # prologue: DVE observes the const DMA queues one at a time, so
            # no later DVE instruction needs more than one semaphore wait
            scr = consts.tile([1, 4], f32, tag="scr")
            nc.vector.tensor_copy(scr[0:1, 0:1], sb_elt[0:1, 0, 0:1])
            scru = consts.tile([1, 4], u32, tag="scru")
            nc.vector.tensor_copy(scru[0:1, 0:1], sb_base[0:1, 0:1])

            sb_eaug = consts.tile([K, EW], bf16, tag="w_eaug")
            nc.vector.tensor_copy(sb_eaug, st_eaug)
            sb_ones = consts.tile([1, K], f32, tag="w_ones")
            nc.vector.tensor_copy(sb_ones, st_ones)
            A = astate.tile([K, NPC], bf16, tag="A")
            nc.vector.tensor_copy(A, st_a0)

            # raw z numerators, shipped to host (z_q at psum row 96)
            zraw = zrows.tile([1, S * NPC], f32)

            ZB = 32  # scan steps sharing one wide PSUM bank
            psumw = [None]

            def crf_step(p):
                # step p (1..256): matmul E_aug^T @ A_{p-1} into column group
                # g=(p-1)%ZB; rows 0:66 feed A_p, row 96 is z_{p-1} numerator
                nonlocal A
                g = (p - 1) % ZB
                if g == 0:
                    pw_new = ps.tile([EW, ZB * NPC], f32, tag="psumw")
                    psumw[0] = pw_new
                pw = psumw[0]
                pcol = pw[:, g * NPC : (g + 1) * NPC]
                nc.tensor.matmul(pcol, sb_eaug, A, start=True, stop=True)
                if p <= S - 1:
                    eL = sb_elt[:, p - 1, :]
                    Anew = astate.tile([K, NPC], bf16, tag="A")
                    if p % RENORM == 0:
                        tinv = small.tile([1, NPC], f32, tag="tinv")
                        nc.vector.reciprocal(
                            tinv, pw[96:97, g * NPC : (g + 1) * NPC]
                        )
                        bc = psb.tile([K, NPC], f32, tag="bc")
                        nc.tensor.matmul(bc, sb_ones, tinv, start=True, stop=True)
                        atmp = small.tile([K, NPC], bf16, tag="atmp")
                        nc.vector.tensor_mul(atmp, pcol[0:K, :], eL)
                        nc.vector.tensor_mul(Anew, atmp, bc)
                    else:
                        nc.vector.tensor_mul(Anew, pcol[0:K, :], eL)
                    A = Anew
                if g == ZB - 1:
                    nc.vector.tensor_copy(
                        zraw[0:1, (p - ZB) * NPC : p * NPC], pw[96:97, :]
                    )

            # ---- argmax over the last axis of log_pa ----
            SEG = 8 * K  # 528: eight 66-segments per max_index call

            def argmax_tile(t):
                xt = xin.tile([128, WTILE * K], f32, tag="xt")
                nc.sync.dma_start(
                    out=xt, in_=lp[t].rearrange("p w k -> p (w k)")
                )
                m = mstats.tile([128, WTILE], f32, tag="m")
                xt3 = xt.rearrange("p (w k) -> p w k", k=K)
                # 8-segment pieces (~0.6us) instead of one 4.4us reduce, so
                # the scheduler can slot them into scan-chain stalls
                for j in range(WTILE // 8):
                    nc.vector.tensor_reduce(
                        m[:, 8 * j : 8 * j + 8],
                        xt3[:, 8 * j : 8 * j + 8, :],
                        axis=mybir.AxisListType.X,
                        op=Alu.max,
                    )
                idx = mstats.tile([128, WTILE], u32, tag="idx")
                for j in range(WTILE // 8):
                    nc.vector.max_index(
                        idx[:, 8 * j : 8 * j + 8],
                        m[:, 8 * j : 8 * j + 8],
                        xt[:, SEG * j : SEG * (j + 1)],
                    )
                pt = preds.tile([128, WTILE], i32, tag="pt")
                nc.vector.tensor_tensor(
                    out=pt, in0=idx, in1=sb_base, op=Alu.subtract
                )
                nc.sync.dma_start(out=pred_o[t], in_=pt)

            # interleave: one argmax tile per 16 scan steps so the static
            # DVE order has filler work during the serial-chain stalls;
            # tile 0 first so its DMA isn't queued behind the constants
            argmax_tile(0)
            for t in range(NTILES):
                for p in range(16 * t + 1, 16 * t + 17):
                    crf_step(p)
                if t + 1 < NTILES:
                    argmax_tile(t + 1)
            nc.sync.dma_start(out=z_o[:], in_=zraw)

    _split_multi_waits(nc)
    return nc


def _get_bass():
    global _CACHED
    if _CACHED is None:
        _CACHED = _build_bass()
    return _CACHED


def kernel(log_pa, score, v_label, v_l, orig_l, role_label,
           start_transitions, transitions, end_transitions):
    log_pa = np.asarray(log_pa, dtype=np.float32)
    score = np.asarray(score, dtype=np.float32)
    v_label = np.asarray(v_label, dtype=np.int32)
    orig_l = np.asarray(orig_l, dtype=np.int32)
    role_label = np.asarray(role_label, dtype=np.int32)
    start_t = np.asarray(start_transitions, dtype=np.float32)
    trans = np.asarray(transitions, dtype=np.float32)
    end_t = np.asarray(end_transitions, dtype=np.float32)

    N = B * V
    # gather per-predicate score rows (host-side shard prep)
    a_score = score[np.arange(B)[:, None], v_label]          # [B,V,S,K]
    a_score = np.ascontiguousarray(a_score.reshape(N, S, K))

    # shared small constants
    eaug = np.zeros((K, 97), np.float32)
    eaug[:, :K] = np.exp(trans)
    eaug[:, 96] = np.exp(end_t)
    eaug = eaug.astype(ml_dtypes.bfloat16)
    ones1 = np.ones((1, K), np.float32)
    # basec[p, w] = K * (w % 8): max_index slot w holds an index local to its
    # 528-wide slice, whose segments are w = 8*j .. 8*j+7 in order
    basec = np.broadcast_to(
        np.tile(np.arange(8, dtype=np.uint32) * K, 8)[None, :], (128, WTILE)
    )
    basec = np.ascontiguousarray(basec)

    in_maps = []
    for c in range(NCORES):
        ns = slice(c * NPC, (c + 1) * NPC)
        lp_c = np.ascontiguousarray(
            log_pa[c * BPC : (c + 1) * BPC].reshape(NTILES, 128, WTILE, K)
        )
        sc = a_score[ns]                                      # [16, S, K]
        elt_c = np.ascontiguousarray(
            np.exp(sc[:, 1:, :] - GSHIFT).transpose(2, 1, 0)  # [K, S-1, 16]
        ).astype(ml_dtypes.bfloat16)
        a0_c = np.ascontiguousarray(
            np.exp(start_t[:, None] + sc[:, 0, :].T)          # [K, 16]
        ).astype(ml_dtypes.bfloat16)
        in_maps.append(
            {
                "lp": lp_c,
                "elt": elt_c,
                "a0": a0_c,
                "eaug": eaug,
                "ones1": ones1,
                "basec": basec,
            }
        )

    nc = _get_bass()
    res = run_bass_kernel_spmd(nc, in_maps, list(range(NCORES)))
    outs = res.results

    # ---- gather/unshard ----
    pred = np.concatenate(
        [outs[c]["pred"].reshape(BPC, S, S) for c in range(NCORES)], axis=0
    ).astype(np.int32)
    zdev = np.concatenate(
        [outs[c]["z"].reshape(S, NPC) for c in range(NCORES)], axis=1
    )  # [S, N] raw z numerators (device scale)

    # patch rare cross-segment max_index collisions exactly
    bad = (pred < 0) | (pred >= K)
    if bad.any():
        bb, bs1, bs2 = np.nonzero(bad)
        for b_, s1_, s2_ in zip(bb, bs1, bs2):
            pred[b_, s1_, s2_] = int(np.argmax(log_pa[b_, s1_, s2_]))

    # ---- host: select log_z at each sequence's last valid position ----
    # device renorm at step p' divides A by the stored zraw[p'-1], so
    # ln z_true(q) = ln zraw[q] + GSHIFT*q + sum_{p'<=q} ln zraw[p'-1]
    L = np.repeat(orig_l, V)                                  # [N]
    q = np.maximum(L - 1, 0).astype(np.int64)
    lnS = np.log(zdev[RENORM - 1 :: RENORM].astype(np.float64))  # [32, N] at q=7,15,..
    lnS = lnS[: (S - RENORM) // RENORM]                       # renorms p'=8..248 → 31
    csum = np.concatenate(
        [np.zeros((1, N), np.float64), np.cumsum(lnS, axis=0)], axis=0
    )
    cnt = np.minimum(q // RENORM, lnS.shape[0])
    ar = np.arange(N)
    log_z = (
        np.log(zdev[q, ar].astype(np.float64))
        + csum[cnt, ar]
        + GSHIFT * q
    ).astype(np.float32)

    # ---- host: gold path score (tiny gathers, mirrors reference) ----
    pos = np.arange(S)
    valid = pos[None, :] < orig_l[:, None]                    # [B,S]
    maskf = np.broadcast_to(valid[:, None, :], (B, V, S)).reshape(N, S).astype(np.float32)
    tags = np.where(valid[:, None, :], role_label, 0).reshape(N, S).astype(np.int32)

    emit = np.take_along_axis(a_score, tags[:, :, None], axis=2)[:, :, 0]  # [N,S]
    trans_sc = trans[tags[:, :-1], tags[:, 1:]]               # [N,S-1]
    gscore = start_t[tags[:, 0]].astype(np.float32)
    gscore = gscore + np.sum(
        emit[:, :-1] * maskf[:, :-1] + trans_sc * maskf[:, 1:], axis=1, dtype=np.float32
    )
    last_idx = np.sum(valid, axis=1).astype(np.int64) - 1     # [B]
    last_tags = np.take_along_axis(
        tags.reshape(B, V, S), np.broadcast_to(last_idx[:, None, None], (B, V, 1)), axis=2
    ).reshape(N)
    gscore = gscore + end_t[last_tags] + emit[:, -1] * maskf[:, -1]

    llh = np.sum(gscore - log_z, dtype=np.float32)
    loss = np.float32(llh / np.float32(N))
    return loss, pred


# revision 44
# speedup vs baseline: 1.0458x; 1.0458x over previous
import sys

sys.path.insert(0, "/opt/trn_rl_repo")

import ml_dtypes
import numpy as np

import concourse.bass as bass
import concourse.mybir as mybir
from concourse.bass_utils import run_bass_kernel_spmd
from concourse.tile import TileContext


# Problem constants (nn_CRFLoss: B,V,S,K = 16,8,256,66 on 8 cores)
B, V, S, K = 16, 8, 256, 66
NCORES = 8
BPC = B // NCORES          # batches per core (2)
NPC = BPC * V              # CRF sequences per core (16)
ROWS = BPC * S * S         # argmax rows per core (131072)
WTILE = 64                 # segments (rows) per partition per argmax tile
NTILES = ROWS // (128 * WTILE)  # 16
GSHIFT = 5.0               # per-step log-scale taken out of the emissions
RENORM = 16                # renormalize the linear recurrence every RENORM steps

_CACHED = None


def _split_multi_waits(nc):
    """This walrus build accepts at most one semaphore wait per compute
    instruction. Move extra waits onto standalone EventSemaphore
    instructions inserted just before, on the same engine stream."""
    n = 0
    for f in nc.m.functions:
        for blk in f.blocks:
            out = []
            for inst in blk.instructions:
                si = getattr(inst, "sync_info", None)
                waits = list(si.on_wait) if si is not None and si.on_wait else []
                if len(waits) > 1:
                    for w in waits[:-1]:
                        ev = mybir.InstEventSemaphore(
                            name=f"I-evsplit-{n}", ins=[], outs=[]
                        )
                        n += 1
                        ev.engine = inst.engine
                        ev.sync_info = mybir.SyncInfo(on_wait=[w], on_update=[])
                        out.append(ev)
                    si.on_wait = waits[-1:]
                out.append(inst)
            blk.instructions = out


def _build_bass():
    f32 = mybir.dt.float32
    u32 = mybir.dt.uint32
    i32 = mybir.dt.int32
    Alu = mybir.AluOpType
    Act = mybir.ActivationFunctionType

    nc = bass.Bass()
    EW = 97  # lhsT width: 66 transition cols, zero pad, exp(end) at col 96
    bf16 = mybir.dt.bfloat16
    lp = nc.declare_dram_parameter("lp", [NTILES, 128, WTILE, K], f32, isOutput=False)
    elt = nc.declare_dram_parameter("elt", [K, S - 1, NPC], bf16, isOutput=False)
    a0 = nc.declare_dram_parameter("a0", [K, NPC], bf16, isOutput=False)
    eaug = nc.declare_dram_parameter("eaug", [K, EW], bf16, isOutput=False)
    ones1 = nc.declare_dram_parameter("ones1", [1, K], f32, isOutput=False)
    basec = nc.declare_dram_parameter("basec", [128, WTILE], u32, isOutput=False)
    pred_o = nc.declare_dram_parameter("pred", [NTILES, 128, WTILE], i32, isOutput=True)
    z_o = nc.declare_dram_parameter("z", [1, S * NPC], f32, isOutput=True)

    with TileContext(nc) as tc:
        with (
            tc.tile_pool(name="consts", bufs=1) as consts,
            tc.tile_pool(name="xin", bufs=3) as xin,
            tc.tile_pool(name="mstats", bufs=3) as mstats,
            tc.tile_pool(name="preds", bufs=3) as preds,
            tc.tile_pool(name="astate", bufs=3) as astate,
            tc.tile_pool(name="small", bufs=4) as small,
            tc.tile_pool(name="zrows", bufs=1) as zrows,
            tc.tile_pool(name="ps", bufs=2, space="PSUM") as ps,
            tc.tile_pool(name="psb", bufs=2, space="PSUM") as psb,
        ):
            # ---- constants ----
            # PE-visible tiles are produced by DVE copies so that every
            # Matmult collapses to a single (DVE) semaphore wait — this
            # walrus build rejects Matmults with more than one wait.
            sb_elt = consts.tile([K, S - 1, NPC], bf16)
            nc.sync.dma_start(out=sb_elt, in_=elt[:])
            sb_base = consts.tile([128, WTILE], u32)
            nc.sync.dma_start(out=sb_base, in_=basec[:])

            st_eaug = consts.tile([K, EW], bf16, tag="st_eaug")
            nc.sync.dma_start(out=st_eaug, in_=eaug[:])
            st_ones = consts.tile([1, K], f32, tag="st_ones")
            nc.sync.dma_start(out=st_ones, in_=ones1[:])
            st_a0 = consts.tile([K, NPC], bf16, tag="st_a0")
            nc.sync.dma_start(out=st_a0, in_=a0[:])

            # prologue: DVE observes the const DMA queues one at a time, so
            # no later DVE instruction needs more than one semaphore wait
            scr = consts.tile([1, 4], f32, tag="scr")
            nc.vector.tensor_copy(scr[0:1, 0:1], sb_elt[0:1, 0, 0:1])
            scru = consts.tile([1, 4], u32, tag="scru")
            nc.vector.tensor_copy(scru[0:1, 0:1], sb_base[0:1, 0:1])

            sb_eaug = consts.tile([K, EW], bf16, tag="w_eaug")
            nc.vector.tensor_copy(sb_eaug, st_eaug)
            sb_ones = consts.tile([1, K], f32, tag="w_ones")
            nc.vector.tensor_copy(sb_ones, st_ones)
            A = astate.tile([K, NPC], bf16, tag="A")
            nc.vector.tensor_copy(A, st_a0)

            # raw z numerators, shipped to host (z_q at psum row 96)
            zraw = zrows.tile([1, S * NPC], f32)

            ZB = 32  # scan steps sharing one wide PSUM bank
            psumw = [None]

            def crf_step(p):
                # step p (1..256): matmul E_aug^T @ A_{p-1} into column group
                # g=(p-1)%ZB; rows 0:66 feed A_p, row 96 is z_{p-1} numerator
                nonlocal A
                g = (p - 1) % ZB
                if g == 0:
                    pw_new = ps.tile([EW, ZB * NPC], f32, tag="psumw")
                    psumw[0] = pw_new
                pw = psumw[0]
                pcol = pw[:, g * NPC : (g + 1) * NPC]
                nc.tensor.matmul(pcol, sb_eaug, A, start=True, stop=True)
                if p <= S - 1:
                    eL = sb_elt[:, p - 1, :]
                    Anew = astate.tile([K, NPC], bf16, tag="A")
                    if p % RENORM == 0:
                        tinv = small.tile([1, NPC], f32, tag="tinv")
                        nc.vector.reciprocal(
                            tinv, pw[96:97, g * NPC : (g + 1) * NPC]
                        )
                        bc = psb.tile([K, NPC], f32, tag="bc")
                        nc.tensor.matmul(bc, sb_ones, tinv, start=True, stop=True)
                        atmp = small.tile([K, NPC], bf16, tag="atmp")
                        nc.vector.tensor_mul(atmp, pcol[0:K, :], eL)
                        nc.vector.tensor_mul(Anew, atmp, bc)
                    else:
                        nc.vector.tensor_mul(Anew, pcol[0:K, :], eL)
                    A = Anew
                if g == ZB - 1:
                    # idle ScalarE takes the z-row extraction off Vector
                    nc.scalar.copy(
                        zraw[0:1, (p - ZB) * NPC : p * NPC], pw[96:97, :]
                    )

            # ---- argmax over the last axis of log_pa ----
            SEG = 8 * K  # 528: eight 66-segments per max_index call

            def argmax_tile(t):
                xt = xin.tile([128, WTILE * K], f32, tag="xt")
                nc.sync.dma_start(
                    out=xt, in_=lp[t].rearrange("p w k -> p (w k)")
                )
                m = mstats.tile([128, WTILE], f32, tag="m")
                xt3 = xt.rearrange("p (w k) -> p w k", k=K)
                # 8-segment pieces (~0.6us) instead of one 4.4us reduce, so
                # the scheduler can slot them into scan-chain stalls
                for j in range(WTILE // 8):
                    nc.vector.tensor_reduce(
                        m[:, 8 * j : 8 * j + 8],
                        xt3[:, 8 * j : 8 * j + 8, :],
                        axis=mybir.AxisListType.X,
                        op=Alu.max,
                    )
                idx = mstats.tile([128, WTILE], u32, tag="idx")
                for j in range(WTILE // 8):
                    nc.vector.max_index(
                        idx[:, 8 * j : 8 * j + 8],
                        m[:, 8 * j : 8 * j + 8],
                        xt[:, SEG * j : SEG * (j + 1)],
                    )
                pt = preds.tile([128, WTILE], i32, tag="pt")
                nc.vector.tensor_tensor(
                    out=pt, in0=idx, in1=sb_base, op=Alu.subtract
                )
                nc.sync.dma_start(out=pred_o[t], in_=pt)

            # interleave: one argmax tile per 16 scan steps so the static
            # DVE order has filler work during the serial-chain stalls
            for t in range(NTILES):
                for p in range(16 * t + 1, 16 * t + 17):
                    crf_step(p)
                argmax_tile(t)
            nc.sync.dma_start(out=z_o[:], in_=zraw)

    _split_multi_waits(nc)
    return nc


def _get_bass():
    global _CACHED
    if _CACHED is None:
        _CACHED = _build_bass()
    return _CACHED


def kernel(log_pa, score, v_label, v_l, orig_l, role_label,
           start_transitions, transitions, end_transitions):
    log_pa = np.asarray(log_pa, dtype=np.float32)
    score = np.asarray(score, dtype=np.float32)
    v_label = np.asarray(v_label, dtype=np.int32)
    orig_l = np.asarray(orig_l, dtype=np.int32)
    role_label = np.asarray(role_label, dtype=np.int32)
    start_t = np.asarray(start_transitions, dtype=np.float32)
    trans = np.asarray(transitions, dtype=np.float32)
    end_t = np.asarray(end_transitions, dtype=np.float32)

    N = B * V
    # gather per-predicate score rows (host-side shard prep)
    a_score = score[np.arange(B)[:, None], v_label]          # [B,V,S,K]
    a_score = np.ascontiguousarray(a_score.reshape(N, S, K))

    # shared small constants
    eaug = np.zeros((K, 97), np.float32)
    eaug[:, :K] = np.exp(trans)
    eaug[:, 96] = np.exp(end_t)
    eaug = eaug.astype(ml_dtypes.bfloat16)
    ones1 = np.ones((1, K), np.float32)
    # basec[p, w] = K * (w % 8): max_index slot w holds an index local to its
    # 528-wide slice, whose segments are w = 8*j .. 8*j+7 in order
    basec = np.broadcast_to(
        np.tile(np.arange(8, dtype=np.uint32) * K, 8)[None, :], (128, WTILE)
    )
    basec = np.ascontiguousarray(basec)

    in_maps = []
    for c in range(NCORES):
        ns = slice(c * NPC, (c + 1) * NPC)
        lp_c = np.ascontiguousarray(
            log_pa[c * BPC : (c + 1) * BPC].reshape(NTILES, 128, WTILE, K)
        )
        sc = a_score[ns]                                      # [16, S, K]
        elt_c = np.ascontiguousarray(
            np.exp(sc[:, 1:, :] - GSHIFT).transpose(2, 1, 0)  # [K, S-1, 16]
        ).astype(ml_dtypes.bfloat16)
        a0_c = np.ascontiguousarray(
            np.exp(start_t[:, None] + sc[:, 0, :].T)          # [K, 16]
        ).astype(ml_dtypes.bfloat16)
        in_maps.append(
            {
                "lp": lp_c,
                "elt": elt_c,
                "a0": a0_c,
                "eaug": eaug,
                "ones1": ones1,
                "basec": basec,
            }
        )

    nc = _get_bass()
    res = run_bass_kernel_spmd(nc, in_maps, list(range(NCORES)))
    outs = res.results

    # ---- gather/unshard ----
    pred = np.concatenate(
        [outs[c]["pred"].reshape(BPC, S, S) for c in range(NCORES)], axis=0
    ).astype(np.int32)
    zdev = np.concatenate(
        [outs[c]["z"].reshape(S, NPC) for c in range(NCORES)], axis=1
    )  # [S, N] raw z numerators (device scale)

    # patch rare cross-segment max_index collisions exactly
    bad = (pred < 0) | (pred >= K)
    if bad.any():
        bb, bs1, bs2 = np.nonzero(bad)
        for b_, s1_, s2_ in zip(bb, bs1, bs2):
            pred[b_, s1_, s2_] = int(np.argmax(log_pa[b_, s1_, s2_]))

    # ---- host: select log_z at each sequence's last valid position ----
    # device renorm at step p' divides A by the stored zraw[p'-1], so
    # ln z_true(q) = ln zraw[q] + GSHIFT*q + sum_{p'<=q} ln zraw[p'-1]
    L = np.repeat(orig_l, V)                                  # [N]
    q = np.maximum(L - 1, 0).astype(np.int64)
    lnS = np.log(zdev[RENORM - 1 :: RENORM].astype(np.float64))  # [32, N] at q=7,15,..
    lnS = lnS[: (S - RENORM) // RENORM]                       # renorms p'=8..248 → 31
    csum = np.concatenate(
        [np.zeros((1, N), np.float64), np.cumsum(lnS, axis=0)], axis=0
    )
    cnt = np.minimum(q // RENORM, lnS.shape[0])
    ar = np.arange(N)
    log_z = (
        np.log(zdev[q, ar].astype(np.float64))
        + csum[cnt, ar]
        + GSHIFT * q
    ).astype(np.float32)

    # ---- host: gold path score (tiny gathers, mirrors reference) ----
    pos = np.arange(S)
    valid = pos[None, :] < orig_l[:, None]                    # [B,S]
    maskf = np.broadcast_to(valid[:, None, :], (B, V, S)).reshape(N, S).astype(np.float32)
    tags = np.where(valid[:, None, :], role_label, 0).reshape(N, S).astype(np.int32)

    emit = np.take_along_axis(a_score, tags[:, :, None], axis=2)[:, :, 0]  # [N,S]
    trans_sc = trans[tags[:, :-1], tags[:, 1:]]               # [N,S-1]
    gscore = start_t[tags[:, 0]].astype(np.float32)
    gscore = gscore + np.sum(
        emit[:, :-1] * maskf[:, :-1] + trans_sc * maskf[:, 1:], axis=1, dtype=np.float32
    )
    last_idx = np.sum(valid, axis=1).astype(np.int64) - 1     # [B]
    last_tags = np.take_along_axis(
        tags.reshape(B, V, S), np.broadcast_to(last_idx[:, None, None], (B, V, 1)), axis=2
    ).reshape(N)
    gscore = gscore + end_t[last_tags] + emit[:, -1] * maskf[:, -1]

    llh = np.sum(gscore - log_z, dtype=np.float32)
    loss = np.float32(llh / np.float32(N))
    return loss, pred
